# revision 28
# baseline (speedup 1.0000x reference)
"""Trainium2 Bass kernel for nn_LocationSlayerRandom (SLAYER two-branch spiking net).

Contract: kernel(**inputs) takes the FULL unsharded inputs
  spike_input [32,156,1,1,2048] f32, W1 [512,156], W2 [20,512],
  Wl1 [512,2048], Wl2 [20,512], perm [156] i32
and returns the FULL output [32,20,1,1,2204] f32.

Strategy (8 cores, data-parallel over batch, 4 samples/core):

Branch 1 (per sample b):
  u1 = psp_t(W1 @ si) = W1 @ psp_t(si)            (psp is linear => commutes)
  - psp_t(si): DVE tensor_tensor_scan along t on the 156-row input
    (channels 0:127 per-b slices of one packed tile; channels 128:155 of all
    4 b packed into one 128-partition tile at offsets 32b, with per-sample
    zero-masked 128-row weight tiles selecting each sample's rows).
  - fc1 on PE (bf16), ACT Sign(u1-10) fused straight from PSUM -> sg in
    {-1,0,1} fp8. fc2 weights are pre-scaled 0.5 (so W2 @ s1 with
    s1 = sg/2 + 1/2); the affine 0.5*rowsum(W2) correction is folded into a
    host-side time-varying threshold T2[o,t] = 10 - 0.5*rowsum(W2_eff)[o]*g[t],
    g[t] = sum_{k<=t} alpha^k.
  - fc2 on PE in fp8, with the four samples packed into the four PE column
    groups (tile_position=(0,32b)) accumulating into ONE [128,2048] PSUM
    tile; one psp scan straight from PSUM; spike_output = (v >= T2).

Branch 2: ul1 = psp_c'(Wl1 @ x_tp) where x_tp[b,t,c'] = si[b,perm[c'],t].
  Host supplies the gathered+transposed input tiles sipT (pure layout prep),
  so the t-contraction runs with Wl1^T stationary and the c'-psp becomes a
  free-dim scan straight from PSUM with a reset-pattern data0 (alpha, but 0
  at each sample boundary). Then threshold, fc2, scan, threshold.

Numerics: matmuls bf16 (fc2-b1 fp8) with fp32 accumulate. The only
nonlinearity is the >=10 threshold; true layer-2 potentials sit below 3.2
(branch 1) / 2.0 (branch 2) against a threshold of 10, so near-threshold
layer-1 bit flips from low-precision weights cannot flip any output bit.
"""

from contextlib import ExitStack

import numpy as np
import ml_dtypes

import concourse.bass as bass
import concourse.mybir as mybir
from concourse import bacc
from concourse import tile as tile_mod
from concourse.bass_utils import run_bass_kernel_spmd

F32 = mybir.dt.float32
BF16 = mybir.dt.bfloat16
FP8 = mybir.dt.float8e4
AL = mybir.AluOpType
AF = mybir.ActivationFunctionType
BF16_NP = ml_dtypes.bfloat16
FP8_NP = ml_dtypes.float8_e4m3

B, C_IN, T = 32, 156, 2048
HID, OUT_DIM = 512, 20
CP = 156                      # permuted taxel axis (branch-2 "time")
N_CORES = 8
B_PER = B // N_CORES          # 4 samples per core
ALPHA = float(np.exp(-1.0 / 10.0))
THETA = 10.0
NB2 = B_PER * CP              # 624, branch-2 packed free dim
KT = T // 128                 # 16 k-tiles over t


def build_program(tc, outs, ins):
    nc = tc.nc
    out = outs["out"]

    with ExitStack() as ctx:
        consts = ctx.enter_context(tc.tile_pool(name="consts", bufs=1))
        work = ctx.enter_context(tc.tile_pool(name="work", bufs=1))
        sgp = ctx.enter_context(tc.tile_pool(name="sgp", bufs=16))
        mid = ctx.enter_context(tc.tile_pool(name="mid", bufs=4))
        psum1 = ctx.enter_context(tc.tile_pool(name="psum1", bufs=2, space="PSUM"))
        psum2 = ctx.enter_context(tc.tile_pool(name="psum2", bufs=1, space="PSUM"))

        # ---------------- constant patterns (gpsimd; SBUF only) ----------
        alpha_t = consts.tile([128, T], F32, tag="alpha")
        nc.gpsimd.memset(alpha_t[:], ALPHA)
        pat624 = consts.tile([128, NB2], F32, tag="pat624")
        nc.gpsimd.memset(pat624[:], ALPHA)
        for j in range(B_PER):
            nc.gpsimd.memset(pat624[:, j * CP:j * CP + 1], 0.0)
        bias_m10 = consts.tile([128, 1], F32, tag="bm10")
        nc.gpsimd.memset(bias_m10[:], -THETA)

        # ---------------- inputs (consolidated DMAs) ---------------------
        # branch-1 critical path first
        siB = consts.tile([128, T], BF16, tag="siB")
        nc.sync.dma_start(siB[:], ins["siB"][:])
        siA = consts.tile([128, B_PER * T], BF16, tag="siA")
        for b in range(B_PER):           # per-sample slices so scan b starts
            nc.sync.dma_start(siA[:, b * T:(b + 1) * T],   # after its own DMA
                              ins["siA"][:, b * T:(b + 1) * T])
        w1a = consts.tile([128, HID], BF16, tag="w1a")
        nc.sync.dma_start(w1a[:], ins["W1Ta"][:])
        w1b = consts.tile([128, B_PER * HID], BF16, tag="w1b")
        nc.sync.dma_start(w1b[:], ins["W1Tb"][:])
        w2p = consts.tile([128, 4 * 32], FP8, tag="w2p")
        nc.sync.dma_start(w2p[:], ins["W2pT"][:])
        t2_t = consts.tile([128, T], F32, tag="t2")
        nc.sync.dma_start(t2_t[:], ins["T2"][:])
        wl1 = consts.tile([128, KT * HID], BF16, tag="wl1")
        nc.sync.dma_start(wl1[:], ins["Wl1T"][:])
        sip = consts.tile([128, KT * NB2], BF16, tag="sip")
        nc.sync.dma_start(sip[:], ins["sipT"][:])
        wl2 = consts.tile([128, 4 * OUT_DIM], BF16, tag="wl2")
        nc.sync.dma_start(wl2[:], ins["Wl2T"][:])

        # ---------------- branch-1 input psp scans (DVE) -----------------
        # order: the packed tail tile first, then sample 0 (fc1 b0 needs both
        # before its first accumulation group completes), then samples 1..3.
        # Emitting the early-needed scans first also keeps their completion
        # events early in the DVE stream (a later event would stall fc1).
        psA = work.tile([128, B_PER * T], BF16, tag="psA")
        psB = work.tile([128, T], BF16, tag="psB")
        # first-needed scans split in half (chained via initial=) so fc1's
        # first accumulation group can start after the first 1024 columns.
        H = T // 2
        nc.vector.tensor_tensor_scan(psB[:, :H], alpha_t[:, :H], siB[:, :H],
                                     0.0, AL.mult, AL.add)
        nc.vector.tensor_tensor_scan(psA[:, :H], alpha_t[:, :H], siA[:, :H],
                                     0.0, AL.mult, AL.add)
        nc.vector.tensor_tensor_scan(psB[:, H:T], alpha_t[:, :H],
                                     siB[:, H:T], psB[:, H - 1:H],
                                     AL.mult, AL.add)
        nc.vector.tensor_tensor_scan(psA[:, H:T], alpha_t[:, :H],
                                     siA[:, H:T], psA[:, H - 1:H],
                                     AL.mult, AL.add)
        for b in range(1, B_PER):
            nc.vector.tensor_tensor_scan(psA[:, b * T:(b + 1) * T], alpha_t[:],
                                         siA[:, b * T:(b + 1) * T], 0.0,
                                         AL.mult, AL.add)

        # ---------------- branch 1 fc1 + fused Sign thresholds -----------
        # loop order b -> half -> m: the whole first inner phase consumes only
        # the first-half scans, so fc1 never stalls on a later half-scan.
        sgt = {}
        for b in range(B_PER):
            for m in range(4):
                sgt[(b, m)] = sgp.tile([128, T], FP8, tag="sg", name=f"sg{b}{m}")
            for half in range(2):
                for m in range(4):
                    s_t = sgt[(b, m)]
                    msl = slice(m * 128, (m + 1) * 128)
                    bmsl = slice(b * HID + m * 128, b * HID + (m + 1) * 128)
                    pu = psum1.tile([128, 1024], F32, tag="psum1")
                    for ch in range(2):
                        tsl = slice(b * T + half * 1024 + ch * 512,
                                    b * T + half * 1024 + (ch + 1) * 512)
                        nc.tensor.matmul(pu[:, ch * 512:(ch + 1) * 512],
                                         w1a[:, msl], psA[:, tsl],
                                         start=True, stop=False)
                    for ch in range(2):
                        tsl = slice(half * 1024 + ch * 512,
                                    half * 1024 + (ch + 1) * 512)
                        nc.tensor.matmul(pu[:, ch * 512:(ch + 1) * 512],
                                         w1b[:, bmsl], psB[:, tsl],
                                         start=False, stop=True)
                    hsl = slice(half * 1024, (half + 1) * 1024)
                    if m < 3:
                        # ACT: sg = sign(u1-10) in {-1,0,1}; weights 0.5x
                        nc.scalar.activation(s_t[:, hsl], pu[:], AF.Sign,
                                             bias=bias_m10[:])
                    else:
                        # DVE: sg = (u1>=10)-0.5 in {-.5,.5}; weights 1.0x
                        nc.vector.tensor_scalar(s_t[:, hsl], pu[:], THETA, 0.5,
                                                AL.is_ge, AL.subtract)

        # ---------------- branch 1 fc2, col-tiled over samples -----------
        pu2 = psum2.tile([128, T], F32, tag="psum2")
        for k in range(4):
            ksl = slice(k * 32, k * 32 + 32)
            for b in range(B_PER):
                for ch in range(4):
                    csl = slice(ch * 512, (ch + 1) * 512)
                    nc.tensor.matmul(pu2[32 * b:32 * b + 32, csl],
                                     w2p[:, ksl], sgt[(b, k)][:, csl],
                                     start=(k == 0), stop=(k == 3),
                                     tile_position=(0, 32 * b),
                                     skip_group_check=True)
        vs = work.tile([128, T], F32, tag="vs")
        nc.vector.tensor_tensor_scan(vs[:], alpha_t[:], pu2[:], 0.0,
                                     AL.mult, AL.add)
        o1 = work.tile([128, T], F32, tag="o1")
        nc.vector.tensor_tensor(o1[:], vs[:], t2_t[:], AL.is_ge)
        # one DMA for all four samples; out is padded to 32 rows per sample
        # (host strips rows 20:32), so the [128, T] tile maps directly.
        nc.sync.dma_start(out[:, :, 0:T].rearrange("b j t -> (b j) t"), o1[:])

        # ---------------- branch 2: A1 = Wl1 @ sipT, psp, threshold ------
        l1 = []
        for m in range(4):
            pa = psum1.tile([128, 1024], F32, tag="psum1")
            a1 = pa[:, :NB2]
            for k in range(KT):
                st, sp = (k == 0), (k == KT - 1)
                wsl = slice(k * HID + m * 128, k * HID + (m + 1) * 128)
                nc.tensor.matmul(a1[:, 0:512], wl1[:, wsl],
                                 sip[:, k * NB2:k * NB2 + 512],
                                 start=st, stop=sp)
                nc.tensor.matmul(a1[:, 512:NB2], wl1[:, wsl],
                                 sip[:, k * NB2 + 512:(k + 1) * NB2],
                                 start=st, stop=sp)
            u = mid.tile([128, NB2], F32, tag="ul1")
            nc.vector.tensor_tensor_scan(u[:], pat624[:], a1, 0.0,
                                         AL.mult, AL.add)
            lt = mid.tile([128, NB2], BF16, tag="l1")
            nc.vector.tensor_scalar(lt[:], u[:], THETA, None, AL.is_ge)
            l1.append(lt)

        # branch 2 fc2 + psp + threshold + out
        pl2full = psum1.tile([128, 1024], F32, tag="psum1")
        pl2 = pl2full[:OUT_DIM, :NB2]
        for k in range(4):
            st, sp = (k == 0), (k == 3)
            ksl = slice(k * OUT_DIM, (k + 1) * OUT_DIM)
            nc.tensor.matmul(pl2[:, 0:512], wl2[:, ksl], l1[k][:, 0:512],
                             start=st, stop=sp)
            nc.tensor.matmul(pl2[:, 512:NB2], wl2[:, ksl], l1[k][:, 512:NB2],
                             start=st, stop=sp)
        ul2 = mid.tile([128, NB2], F32, tag="ul2")
        nc.vector.tensor_tensor_scan(ul2[:OUT_DIM], pat624[:OUT_DIM], pl2, 0.0,
                                     AL.mult, AL.add)
        o2 = mid.tile([128, NB2], F32, tag="o2")
        nc.vector.tensor_scalar(o2[:OUT_DIM], ul2[:OUT_DIM], THETA, None,
                                AL.is_ge)
        nc.sync.dma_start(
            out[:, :OUT_DIM, T:T + CP].rearrange("b o c -> o b c"),
            o2[:OUT_DIM, :].rearrange("o (b c) -> o b c", c=CP))


# ======================= host-side preparation =======================

def prep_core_inputs(si, sip, core):
    """Per-core data tensors, pre-packed into single-DMA SBUF layouts.
    si/sip are [32,156,2048] f32 (sip already perm-gathered)."""
    sl = si[core * B_PER:(core + 1) * B_PER]          # [4,156,2048]
    # siA [128, 4*T]: [p, b*T+t] = si[b, p, t]
    siA = np.ascontiguousarray(
        sl[:, :128, :].transpose(1, 0, 2).reshape(128, B_PER * T)
    ).astype(BF16_NP)
    siB = np.zeros((128, T), dtype=BF16_NP)
    for b in range(B_PER):
        siB[32 * b:32 * b + (C_IN - 128)] = sl[b, 128:C_IN, :]
    sp = sip[core * B_PER:(core + 1) * B_PER]         # [4,156,2048]
    # sipT [128, KT*NB2]: [p, k*NB2 + b*CP + c'] = sip[b, c', 128k+p]
    sipT = np.ascontiguousarray(
        sp.transpose(2, 0, 1).reshape(KT, 128, NB2)
        .transpose(1, 0, 2).reshape(128, KT * NB2)
    ).astype(BF16_NP)
    return {"siA": siA, "siB": siB, "sipT": sipT}


def prep_shared_inputs(W1, W2, Wl1, Wl2):
    """Weight layouts + threshold tensor, shared by all cores."""
    w1t = np.zeros((160, HID), dtype=np.float32)
    w1t[:C_IN] = W1.T
    W1Ta = w1t[:128].astype(BF16_NP)
    # Tail channels 128:155 live at partitions 32b..32b+27 of the packed psB
    # tile; per-sample 128-row weight tiles, zero outside the sample's rows,
    # packed [128, 4*HID] with [p, b*HID+o].
    W1Tb = np.zeros((128, B_PER * HID), dtype=BF16_NP)
    for b in range(B_PER):
        W1Tb[32 * b:32 * b + 32, b * HID:(b + 1) * HID] = \
            w1t[128:160].astype(BF16_NP)

    # fc2 weights, fp8, padded to 32 cols per k-tile so the col-tiled
    # matmuls initialize full 32-row PSUM stripes. Per-k scale matches the
    # sg encoding of hidden m-tile k: ACT Sign (+-1) -> 0.5x, DVE (+-.5)
    # -> 1.0x. Layout [128, 4*32]: [p, k*32+o]
    k_scale = (0.5, 0.5, 0.5, 1.0)
    w2t = W2.T.astype(np.float32)                     # [512, 20]
    W2pT = np.zeros((128, 4 * 32), dtype=FP8_NP)
    for k in range(4):
        W2pT[:, k * 32:k * 32 + OUT_DIM] = (
            k_scale[k] * w2t[k * 128:(k + 1) * 128]).astype(FP8_NP)
    # effective (device) W2 after fp8 rounding, unscaled
    w2_eff = np.empty((HID, OUT_DIM), dtype=np.float32)
    for k in range(4):
        w2_eff[k * 128:(k + 1) * 128] = (
            W2pT[:, k * 32:k * 32 + OUT_DIM].astype(np.float32) / k_scale[k]
        )
    r2 = w2_eff.sum(axis=0)                           # [20]
    g = (1.0 - ALPHA ** (np.arange(T, dtype=np.float64) + 1)) / (1.0 - ALPHA)
    theta2 = (THETA - 0.5 * np.outer(r2, g)).astype(np.float32)   # [20, T]
    T2 = np.full((128, T), 1e30, dtype=np.float32)
    for b in range(B_PER):
        T2[32 * b:32 * b + OUT_DIM] = theta2

    # Wl1T [128, KT*HID]: [p, k*HID+o] = Wl1[o, 128k+p]
    Wl1T = np.ascontiguousarray(
        Wl1.T.reshape(KT, 128, HID).transpose(1, 0, 2).reshape(128, KT * HID)
    ).astype(BF16_NP)
    # Wl2T [128, 4*OUT]: [p, k*OUT+o] = Wl2[o, 128k+p]
    Wl2T = np.ascontiguousarray(
        Wl2.T.reshape(4, 128, OUT_DIM).transpose(1, 0, 2).reshape(128, 4 * OUT_DIM)
    ).astype(BF16_NP)
    return {"W1Ta": W1Ta, "W1Tb": W1Tb, "W2pT": W2pT, "Wl1T": Wl1T,
            "Wl2T": Wl2T, "T2": T2}


def make_in_maps(spike_input, W1, W2, Wl1, Wl2, perm):
    si = np.asarray(spike_input, dtype=np.float32).reshape(B, C_IN, T)
    perm = np.asarray(perm).astype(np.int64)
    sip = si[:, perm, :]                              # perm-gather (layout only)
    shared = prep_shared_inputs(np.asarray(W1, np.float32),
                                np.asarray(W2, np.float32),
                                np.asarray(Wl1, np.float32),
                                np.asarray(Wl2, np.float32))
    in_maps = []
    for core in range(N_CORES):
        m = dict(shared)
        m.update(prep_core_inputs(si, sip, core))
        in_maps.append(m)
    return in_maps


_IN_SPECS = {
    "siA": ((128, B_PER * T), BF16),
    "siB": ((128, T), BF16),
    "sipT": ((128, KT * NB2), BF16),
    "W1Ta": ((128, HID), BF16),
    "W1Tb": ((128, B_PER * HID), BF16),
    "W2pT": ((128, 4 * 32), FP8),
    "Wl1T": ((128, KT * HID), BF16),
    "Wl2T": ((128, 4 * OUT_DIM), BF16),
    "T2": ((128, T), F32),
}


def build_bass():
    nc = bacc.Bacc("TRN2", target_bir_lowering=False, debug=False)
    ins = {}
    for name, (shape, dt) in _IN_SPECS.items():
        h = nc.dram_tensor(name, list(shape), dt, kind="ExternalInput")
        ins[name] = h[:]
    out_h = nc.dram_tensor("out", [B_PER, 32, T + CP], F32,
                           kind="ExternalOutput")
    outs = {"out": out_h[:]}
    with tile_mod.TileContext(nc) as tc:
        build_program(tc, outs, ins)
    nc.compile()
    return nc


_NC_CACHE = None


def run(inputs, trace=False, **kw):
    """Run on the 8 NeuronCores; returns (full_output, BassKernelResults)."""
    global _NC_CACHE
    if _NC_CACHE is None:
        _NC_CACHE = build_bass()
    nc = _NC_CACHE
    in_maps = make_in_maps(**inputs)
    res = run_bass_kernel_spmd(nc, in_maps, core_ids=list(range(N_CORES)),
                               trace=trace, **kw)
    parts = [res.results[c]["out"][:, :OUT_DIM, :] for c in range(N_CORES)]
    full = np.concatenate(parts, axis=0).reshape(B, OUT_DIM, 1, 1, T + CP)
    return np.ascontiguousarray(full.astype(np.float32)), res


def kernel(**inputs):
    out, _ = run(inputs)
    return out


# revision 29
# speedup vs baseline: 1.0071x; 1.0071x over previous
"""Trainium2 Bass kernel for nn_LocationSlayerRandom (SLAYER two-branch spiking net).

Contract: kernel(**inputs) takes the FULL unsharded inputs
  spike_input [32,156,1,1,2048] f32, W1 [512,156], W2 [20,512],
  Wl1 [512,2048], Wl2 [20,512], perm [156] i32
and returns the FULL output [32,20,1,1,2204] f32.

Strategy (8 cores, data-parallel over batch, 4 samples/core):

Branch 1 (per sample b):
  u1 = psp_t(W1 @ si) = W1 @ psp_t(si)            (psp is linear => commutes)
  - psp_t(si): DVE tensor_tensor_scan along t on the 156-row input
    (channels 0:127 per-b slices of one packed tile; channels 128:155 of all
    4 b packed into one 128-partition tile at offsets 32b, with per-sample
    zero-masked 128-row weight tiles selecting each sample's rows).
  - fc1 on PE (bf16), ACT Sign(u1-10) fused straight from PSUM -> sg in
    {-1,0,1} fp8. fc2 weights are pre-scaled 0.5 (so W2 @ s1 with
    s1 = sg/2 + 1/2); the affine 0.5*rowsum(W2) correction is folded into a
    host-side time-varying threshold T2[o,t] = 10 - 0.5*rowsum(W2_eff)[o]*g[t],
    g[t] = sum_{k<=t} alpha^k.
  - fc2 on PE in fp8, with the four samples packed into the four PE column
    groups (tile_position=(0,32b)) accumulating into ONE [128,2048] PSUM
    tile; one psp scan straight from PSUM; spike_output = (v >= T2).

Branch 2: ul1 = psp_c'(Wl1 @ x_tp) where x_tp[b,t,c'] = si[b,perm[c'],t].
  Host supplies the gathered+transposed input tiles sipT (pure layout prep),
  so the t-contraction runs with Wl1^T stationary and the c'-psp becomes a
  free-dim scan straight from PSUM with a reset-pattern data0 (alpha, but 0
  at each sample boundary). Then threshold, fc2, scan, threshold.

Numerics: matmuls bf16 (fc2-b1 fp8) with fp32 accumulate. The only
nonlinearity is the >=10 threshold; true layer-2 potentials sit below 3.2
(branch 1) / 2.0 (branch 2) against a threshold of 10, so near-threshold
layer-1 bit flips from low-precision weights cannot flip any output bit.
"""

from contextlib import ExitStack

import numpy as np
import ml_dtypes

import concourse.bass as bass
import concourse.mybir as mybir
from concourse import bacc
from concourse import tile as tile_mod
from concourse.bass_utils import run_bass_kernel_spmd

F32 = mybir.dt.float32
BF16 = mybir.dt.bfloat16
FP8 = mybir.dt.float8e4
AL = mybir.AluOpType
AF = mybir.ActivationFunctionType
BF16_NP = ml_dtypes.bfloat16
FP8_NP = ml_dtypes.float8_e4m3

B, C_IN, T = 32, 156, 2048
HID, OUT_DIM = 512, 20
CP = 156                      # permuted taxel axis (branch-2 "time")
N_CORES = 8
B_PER = B // N_CORES          # 4 samples per core
ALPHA = float(np.exp(-1.0 / 10.0))
THETA = 10.0
NB2 = B_PER * CP              # 624, branch-2 packed free dim
KT = T // 128                 # 16 k-tiles over t


def build_program(tc, outs, ins):
    nc = tc.nc
    out = outs["out"]

    with ExitStack() as ctx:
        consts = ctx.enter_context(tc.tile_pool(name="consts", bufs=1))
        work = ctx.enter_context(tc.tile_pool(name="work", bufs=1))
        sgp = ctx.enter_context(tc.tile_pool(name="sgp", bufs=16))
        mid = ctx.enter_context(tc.tile_pool(name="mid", bufs=4))
        psum1 = ctx.enter_context(tc.tile_pool(name="psum1", bufs=2, space="PSUM"))
        psum2 = ctx.enter_context(tc.tile_pool(name="psum2", bufs=1, space="PSUM"))

        # ---------------- constant patterns (gpsimd; SBUF only) ----------
        alpha_t = consts.tile([128, T], F32, tag="alpha")
        nc.gpsimd.memset(alpha_t[:], ALPHA)
        pat624 = consts.tile([128, NB2], F32, tag="pat624")
        nc.gpsimd.memset(pat624[:], ALPHA)
        for j in range(B_PER):
            nc.gpsimd.memset(pat624[:, j * CP:j * CP + 1], 0.0)
        bias_m10 = consts.tile([128, 1], F32, tag="bm10")
        nc.gpsimd.memset(bias_m10[:], -THETA)

        # ---------------- inputs (consolidated DMAs) ---------------------
        # branch-1 critical path first
        siB = consts.tile([128, T], BF16, tag="siB")
        nc.sync.dma_start(siB[:], ins["siB"][:])
        siA = consts.tile([128, B_PER * T], BF16, tag="siA")
        for b in range(B_PER):           # per-sample slices so scan b starts
            nc.sync.dma_start(siA[:, b * T:(b + 1) * T],   # after its own DMA
                              ins["siA"][:, b * T:(b + 1) * T])
        w1a = consts.tile([128, HID], BF16, tag="w1a")
        nc.sync.dma_start(w1a[:], ins["W1Ta"][:])
        w1b = consts.tile([128, B_PER * HID], BF16, tag="w1b")
        nc.sync.dma_start(w1b[:], ins["W1Tb"][:])
        w2p = consts.tile([128, 4 * 32], FP8, tag="w2p")
        nc.sync.dma_start(w2p[:], ins["W2pT"][:])
        t2_t = consts.tile([128, T], F32, tag="t2")
        nc.sync.dma_start(t2_t[:], ins["T2"][:])
        wl1 = consts.tile([128, KT * HID], BF16, tag="wl1")
        nc.sync.dma_start(wl1[:], ins["Wl1T"][:])
        sip = consts.tile([128, KT * NB2], BF16, tag="sip")
        nc.sync.dma_start(sip[:], ins["sipT"][:])
        wl2 = consts.tile([128, 4 * OUT_DIM], BF16, tag="wl2")
        nc.sync.dma_start(wl2[:], ins["Wl2T"][:])

        # ---------------- branch-1 input psp scans (DVE) -----------------
        # order: the packed tail tile first, then sample 0 (fc1 b0 needs both
        # before its first accumulation group completes), then samples 1..3.
        # Emitting the early-needed scans first also keeps their completion
        # events early in the DVE stream (a later event would stall fc1).
        psA = work.tile([128, B_PER * T], BF16, tag="psA")
        psB = work.tile([128, T], BF16, tag="psB")
        nc.vector.tensor_tensor_scan(psB[:], alpha_t[:], siB[:], 0.0,
                                     AL.mult, AL.add)
        for b in range(B_PER):
            nc.vector.tensor_tensor_scan(psA[:, b * T:(b + 1) * T], alpha_t[:],
                                         siA[:, b * T:(b + 1) * T], 0.0,
                                         AL.mult, AL.add)

        # ---------------- branch 1 fc1 + fused Sign thresholds -----------
        # loop order b -> half -> m: the whole first inner phase consumes only
        # the first-half scans, so fc1 never stalls on a later half-scan.
        sgt = {}
        for b in range(B_PER):
            for m in range(4):
                sgt[(b, m)] = sgp.tile([128, T], FP8, tag="sg", name=f"sg{b}{m}")
            for half in range(2):
                for m in range(4):
                    s_t = sgt[(b, m)]
                    msl = slice(m * 128, (m + 1) * 128)
                    bmsl = slice(b * HID + m * 128, b * HID + (m + 1) * 128)
                    pu = psum1.tile([128, 1024], F32, tag="psum1")
                    for ch in range(2):
                        tsl = slice(b * T + half * 1024 + ch * 512,
                                    b * T + half * 1024 + (ch + 1) * 512)
                        nc.tensor.matmul(pu[:, ch * 512:(ch + 1) * 512],
                                         w1a[:, msl], psA[:, tsl],
                                         start=True, stop=False)
                    for ch in range(2):
                        tsl = slice(half * 1024 + ch * 512,
                                    half * 1024 + (ch + 1) * 512)
                        nc.tensor.matmul(pu[:, ch * 512:(ch + 1) * 512],
                                         w1b[:, bmsl], psB[:, tsl],
                                         start=False, stop=True)
                    hsl = slice(half * 1024, (half + 1) * 1024)
                    if m < 3:
                        # ACT: sg = sign(u1-10) in {-1,0,1}; weights 0.5x
                        nc.scalar.activation(s_t[:, hsl], pu[:], AF.Sign,
                                             bias=bias_m10[:])
                    else:
                        # DVE: sg = (u1>=10)-0.5 in {-.5,.5}; weights 1.0x
                        nc.vector.tensor_scalar(s_t[:, hsl], pu[:], THETA, 0.5,
                                                AL.is_ge, AL.subtract)

        # ---------------- branch 1 fc2, col-tiled over samples -----------
        pu2 = psum2.tile([128, T], F32, tag="psum2")
        for k in range(4):
            ksl = slice(k * 32, k * 32 + 32)
            for b in range(B_PER):
                for ch in range(4):
                    csl = slice(ch * 512, (ch + 1) * 512)
                    nc.tensor.matmul(pu2[32 * b:32 * b + 32, csl],
                                     w2p[:, ksl], sgt[(b, k)][:, csl],
                                     start=(k == 0), stop=(k == 3),
                                     tile_position=(0, 32 * b),
                                     skip_group_check=True)
        vs = work.tile([128, T], F32, tag="vs")
        nc.vector.tensor_tensor_scan(vs[:], alpha_t[:], pu2[:], 0.0,
                                     AL.mult, AL.add)
        o1 = work.tile([128, T], F32, tag="o1")
        nc.vector.tensor_tensor(o1[:], vs[:], t2_t[:], AL.is_ge)
        # one DMA for all four samples; out is padded to 32 rows per sample
        # (host strips rows 20:32), so the [128, T] tile maps directly.
        nc.sync.dma_start(out[:, :, 0:T].rearrange("b j t -> (b j) t"), o1[:])

        # ---------------- branch 2: A1 = Wl1 @ sipT, psp, threshold ------
        l1 = []
        for m in range(4):
            pa = psum1.tile([128, 1024], F32, tag="psum1")
            a1 = pa[:, :NB2]
            for k in range(KT):
                st, sp = (k == 0), (k == KT - 1)
                wsl = slice(k * HID + m * 128, k * HID + (m + 1) * 128)
                nc.tensor.matmul(a1[:, 0:512], wl1[:, wsl],
                                 sip[:, k * NB2:k * NB2 + 512],
                                 start=st, stop=sp)
                nc.tensor.matmul(a1[:, 512:NB2], wl1[:, wsl],
                                 sip[:, k * NB2 + 512:(k + 1) * NB2],
                                 start=st, stop=sp)
            u = mid.tile([128, NB2], F32, tag="ul1")
            nc.vector.tensor_tensor_scan(u[:], pat624[:], a1, 0.0,
                                         AL.mult, AL.add)
            lt = mid.tile([128, NB2], BF16, tag="l1")
            nc.vector.tensor_scalar(lt[:], u[:], THETA, None, AL.is_ge)
            l1.append(lt)

        # branch 2 fc2 + psp + threshold + out
        pl2full = psum1.tile([128, 1024], F32, tag="psum1")
        pl2 = pl2full[:OUT_DIM, :NB2]
        for k in range(4):
            st, sp = (k == 0), (k == 3)
            ksl = slice(k * OUT_DIM, (k + 1) * OUT_DIM)
            nc.tensor.matmul(pl2[:, 0:512], wl2[:, ksl], l1[k][:, 0:512],
                             start=st, stop=sp)
            nc.tensor.matmul(pl2[:, 512:NB2], wl2[:, ksl], l1[k][:, 512:NB2],
                             start=st, stop=sp)
        ul2 = mid.tile([128, NB2], F32, tag="ul2")
        nc.vector.tensor_tensor_scan(ul2[:OUT_DIM], pat624[:OUT_DIM], pl2, 0.0,
                                     AL.mult, AL.add)
        o2 = mid.tile([128, NB2], F32, tag="o2")
        nc.vector.tensor_scalar(o2[:OUT_DIM], ul2[:OUT_DIM], THETA, None,
                                AL.is_ge)
        nc.sync.dma_start(
            out[:, :OUT_DIM, T:T + CP].rearrange("b o c -> o b c"),
            o2[:OUT_DIM, :].rearrange("o (b c) -> o b c", c=CP))


# ======================= host-side preparation =======================

def prep_core_inputs(si, sip, core):
    """Per-core data tensors, pre-packed into single-DMA SBUF layouts.
    si/sip are [32,156,2048] f32 (sip already perm-gathered)."""
    sl = si[core * B_PER:(core + 1) * B_PER]          # [4,156,2048]
    # siA [128, 4*T]: [p, b*T+t] = si[b, p, t]
    siA = np.ascontiguousarray(
        sl[:, :128, :].transpose(1, 0, 2).reshape(128, B_PER * T)
    ).astype(BF16_NP)
    siB = np.zeros((128, T), dtype=BF16_NP)
    for b in range(B_PER):
        siB[32 * b:32 * b + (C_IN - 128)] = sl[b, 128:C_IN, :]
    sp = sip[core * B_PER:(core + 1) * B_PER]         # [4,156,2048]
    # sipT [128, KT*NB2]: [p, k*NB2 + b*CP + c'] = sip[b, c', 128k+p]
    sipT = np.ascontiguousarray(
        sp.transpose(2, 0, 1).reshape(KT, 128, NB2)
        .transpose(1, 0, 2).reshape(128, KT * NB2)
    ).astype(BF16_NP)
    return {"siA": siA, "siB": siB, "sipT": sipT}


def prep_shared_inputs(W1, W2, Wl1, Wl2):
    """Weight layouts + threshold tensor, shared by all cores."""
    w1t = np.zeros((160, HID), dtype=np.float32)
    w1t[:C_IN] = W1.T
    W1Ta = w1t[:128].astype(BF16_NP)
    # Tail channels 128:155 live at partitions 32b..32b+27 of the packed psB
    # tile; per-sample 128-row weight tiles, zero outside the sample's rows,
    # packed [128, 4*HID] with [p, b*HID+o].
    W1Tb = np.zeros((128, B_PER * HID), dtype=BF16_NP)
    for b in range(B_PER):
        W1Tb[32 * b:32 * b + 32, b * HID:(b + 1) * HID] = \
            w1t[128:160].astype(BF16_NP)

    # fc2 weights, fp8, padded to 32 cols per k-tile so the col-tiled
    # matmuls initialize full 32-row PSUM stripes. Per-k scale matches the
    # sg encoding of hidden m-tile k: ACT Sign (+-1) -> 0.5x, DVE (+-.5)
    # -> 1.0x. Layout [128, 4*32]: [p, k*32+o]
    k_scale = (0.5, 0.5, 0.5, 1.0)
    w2t = W2.T.astype(np.float32)                     # [512, 20]
    W2pT = np.zeros((128, 4 * 32), dtype=FP8_NP)
    for k in range(4):
        W2pT[:, k * 32:k * 32 + OUT_DIM] = (
            k_scale[k] * w2t[k * 128:(k + 1) * 128]).astype(FP8_NP)
    # effective (device) W2 after fp8 rounding, unscaled
    w2_eff = np.empty((HID, OUT_DIM), dtype=np.float32)
    for k in range(4):
        w2_eff[k * 128:(k + 1) * 128] = (
            W2pT[:, k * 32:k * 32 + OUT_DIM].astype(np.float32) / k_scale[k]
        )
    r2 = w2_eff.sum(axis=0)                           # [20]
    g = (1.0 - ALPHA ** (np.arange(T, dtype=np.float64) + 1)) / (1.0 - ALPHA)
    theta2 = (THETA - 0.5 * np.outer(r2, g)).astype(np.float32)   # [20, T]
    T2 = np.full((128, T), 1e30, dtype=np.float32)
    for b in range(B_PER):
        T2[32 * b:32 * b + OUT_DIM] = theta2

    # Wl1T [128, KT*HID]: [p, k*HID+o] = Wl1[o, 128k+p]
    Wl1T = np.ascontiguousarray(
        Wl1.T.reshape(KT, 128, HID).transpose(1, 0, 2).reshape(128, KT * HID)
    ).astype(BF16_NP)
    # Wl2T [128, 4*OUT]: [p, k*OUT+o] = Wl2[o, 128k+p]
    Wl2T = np.ascontiguousarray(
        Wl2.T.reshape(4, 128, OUT_DIM).transpose(1, 0, 2).reshape(128, 4 * OUT_DIM)
    ).astype(BF16_NP)
    return {"W1Ta": W1Ta, "W1Tb": W1Tb, "W2pT": W2pT, "Wl1T": Wl1T,
            "Wl2T": Wl2T, "T2": T2}


def make_in_maps(spike_input, W1, W2, Wl1, Wl2, perm):
    si = np.asarray(spike_input, dtype=np.float32).reshape(B, C_IN, T)
    perm = np.asarray(perm).astype(np.int64)
    sip = si[:, perm, :]                              # perm-gather (layout only)
    shared = prep_shared_inputs(np.asarray(W1, np.float32),
                                np.asarray(W2, np.float32),
                                np.asarray(Wl1, np.float32),
                                np.asarray(Wl2, np.float32))
    in_maps = []
    for core in range(N_CORES):
        m = dict(shared)
        m.update(prep_core_inputs(si, sip, core))
        in_maps.append(m)
    return in_maps


_IN_SPECS = {
    "siA": ((128, B_PER * T), BF16),
    "siB": ((128, T), BF16),
    "sipT": ((128, KT * NB2), BF16),
    "W1Ta": ((128, HID), BF16),
    "W1Tb": ((128, B_PER * HID), BF16),
    "W2pT": ((128, 4 * 32), FP8),
    "Wl1T": ((128, KT * HID), BF16),
    "Wl2T": ((128, 4 * OUT_DIM), BF16),
    "T2": ((128, T), F32),
}


def build_bass():
    nc = bacc.Bacc("TRN2", target_bir_lowering=False, debug=False)
    ins = {}
    for name, (shape, dt) in _IN_SPECS.items():
        h = nc.dram_tensor(name, list(shape), dt, kind="ExternalInput")
        ins[name] = h[:]
    out_h = nc.dram_tensor("out", [B_PER, 32, T + CP], F32,
                           kind="ExternalOutput")
    outs = {"out": out_h[:]}
    with tile_mod.TileContext(nc) as tc:
        build_program(tc, outs, ins)
    nc.compile()
    return nc


_NC_CACHE = None


def run(inputs, trace=False, **kw):
    """Run on the 8 NeuronCores; returns (full_output, BassKernelResults)."""
    global _NC_CACHE
    if _NC_CACHE is None:
        _NC_CACHE = build_bass()
    nc = _NC_CACHE
    in_maps = make_in_maps(**inputs)
    res = run_bass_kernel_spmd(nc, in_maps, core_ids=list(range(N_CORES)),
                               trace=trace, **kw)
    parts = [res.results[c]["out"][:, :OUT_DIM, :] for c in range(N_CORES)]
    full = np.concatenate(parts, axis=0).reshape(B, OUT_DIM, 1, 1, T + CP)
    return np.ascontiguousarray(full.astype(np.float32)), res


def kernel(**inputs):
    out, _ = run(inputs)
    return out
